# revision 31
# baseline (speedup 1.0000x reference)
"""Boundary-loss kernel for Trainium2 (8 NeuronCores, pure data parallel).

Computes mean(phi_G * sigmoid(predictions)) where phi_G is the per-sample
normalized signed EDT of the target mask, via phi = (1-2t) * u2d with
u2d = distance to the nearest opposite-class pixel:

    u2d(r,c)^2 = min( (ucol+1)^2,  min_{|k|<K} (hrow(r+k,c)+1)^2 + k^2 )

hrow / ucol are exact 1-D opposite-distances (minus 1) along rows /
columns, from tensor_tensor_scan passes over equality fields.  Only the
vertical parabola is windowed (K), certified by max(E) <= K^2.

v2 engine plan (Pool engine supports only memset/iota/affine_select and
cross-partition reduces through this toolchain):
  DVE   : eq fields, all 6 scans, merges, ring mins, signed dump
  ACT   : casts, tbT PSUM->SBUF copy, sigmoid, squares, q=p2+k^2 biased
          copies, sqrt
  PE    : mask transpose, hrow transpose, sigmoid transpose (no DMA
          transposes at all)
  Pool  : memsets + cross-partition output folds (single-descriptor out)
  DMA   : inputs split across both HWDGE queues; output is [1,8] from
          partition 0 (one descriptor).
"""

import numpy as np
from contextlib import ExitStack

import concourse.bass as bass
import concourse.tile as tile
from concourse import bacc, mybir, masks, bass_isa
from concourse.bass_utils import run_bass_kernel_spmd

B, C, H, W = 8, 1, 256, 256
P = 128
NCH = 2
BIG = 300.0
PADV = 60000.0
YW = 2 * 256 + 1          # 513

Alu = mybir.AluOpType
Act = mybir.ActivationFunctionType
F32 = mybir.dt.float32
BF16 = mybir.dt.bfloat16
I32 = mybir.dt.int32

_K_LADDER = [3, 7, 15, 31, 63, 127, 255]
POOL_FOLDS = True         # gpsimd cross-partition reduces for the output


def _seg2(ap_tile, start, segstr, width=256):
    return (ap_tile[:, start:start + 2 * segstr]
            .rearrange("p (s t) -> p s t", s=2)[:, :, 0:width])


def _segw(ap_tile, start, segstr, w, width=256):
    return ap_tile[:, start + w * segstr: start + w * segstr + width]


def _kernel_body(ctx: ExitStack, tc, out_ap, tgt_ap, pred_ap, K: int):
    nc = tc.nc
    use_bf16 = K <= 11
    dt_e = BF16 if use_bf16 else F32

    SEGSTR = 256 + 2 * K + 2
    LP = K + 1
    PW = LP + 2 * SEGSTR + K + 2

    pool = ctx.enter_context(tc.tile_pool(name="work", bufs=1))
    psum = ctx.enter_context(tc.tile_pool(name="ps", bufs=1, space="PSUM"))

    # ---------------- input DMAs (halves on both HWDGE queues) --------
    T = pool.tile([P, NCH, 256], I32, tag="T")
    Pt = pool.tile([P, NCH, 256], F32, tag="Pt")
    # row-pair layout: partition p holds rows 2p (c=0) and 2p+1 (c=1), so
    # each partition's 2KB is CONTIGUOUS in HBM -> 2KB DMA descriptors
    # (half the descriptor count of the row-banded layout).  Split by
    # partition-half across both HWDGE queues.
    tgt_pc = tgt_ap.rearrange("(p c) j -> p c j", c=NCH)
    pred_pc = pred_ap.rearrange("(p c) j -> p c j", c=NCH)
    nc.sync.dma_start(T[0:64], tgt_pc[0:64])
    nc.scalar.dma_start(T[64:P], tgt_pc[64:P])
    nc.sync.dma_start(Pt[0:64], pred_pc[0:64])
    nc.scalar.dma_start(Pt[64:P], pred_pc[64:P])

    # ---------------- constants (Pool) --------------------------------
    dummy1 = pool.tile([1, 8], F32, tag="dm1")
    dummy2 = pool.tile([1, 8], F32, tag="dm2")
    nc.gpsimd.memset(dummy1[:], 0.5)
    ONEB = pool.tile([P, YW], BF16, tag="ONEB")
    nc.gpsimd.memset(ONEB[:], 1.0)
    nc.gpsimd.memset(ONEB[:, 256:257], BIG)
    YA = pool.tile([P, YW], BF16, tag="YA")
    nc.gpsimd.memset(YA[:], 1.0)
    YB = pool.tile([P, YW], BF16, tag="YB")
    nc.gpsimd.memset(YB[:], 1.0)
    p2 = pool.tile([P, PW], dt_e, tag="p2")
    nc.gpsimd.memset(p2[:], PADV)
    ident = pool.tile([P, P], BF16, tag="ident")
    masks.make_identity(nc, ident[:])

    # ---------------- casts (ACT) -------------------------------------
    tb = pool.tile([P, NCH, 256], BF16, tag="tb")
    nc.scalar.activation(tb[:, 0, :], T[:, 0, :], Act.Copy)
    nc.scalar.activation(tb[:, 1, :], T[:, 1, :], Act.Copy)

    # ---------------- chain A equality + scans (DVE) ------------------
    for c in range(NCH):
        nc.vector.tensor_tensor(YA[:, 1 + 256 * c:256 * c + 256],
                                T[:, c, 0:255], T[:, c, 1:256],
                                op=Alu.is_equal)
    FA = pool.tile([P, YW], BF16, tag="FA")
    BA = pool.tile([P, YW + 1], BF16, tag="BA")
    for c in range(NCH):
        lo = 256 * c
        nc.vector.tensor_tensor_scan(
            out=FA[:, lo:lo + 256], data0=ONEB[:, lo:lo + 256],
            data1=YA[:, lo:lo + 256], initial=BIG, op0=Alu.add, op1=Alu.mult)
        nc.vector.tensor_tensor_scan(
            out=BA[:, lo + 257:lo + 1:-1], data0=ONEB[:, lo + 256:lo:-1],
            data1=YA[:, lo + 256:lo:-1], initial=BIG, op0=Alu.add,
            op1=Alu.mult)
    U = pool.tile([P, NCH, 256], BF16, tag="U")
    nc.vector.tensor_tensor(U[:], _seg2(FA, 0, 256), _seg2(BA, 2, 256),
                            op=Alu.min)

    # With the row-pair input layout, a transposed block (c,w) holds rows
    # 2*j+c along its free dim.  PSUM keeps blocks contiguous at [c, w*128+j];
    # the PSUM->SBUF scatter restores true row order via stride-2 writes.
    def _il(t):
        # psum tile [P, c, w*128+j] -> (c, w, j) view
        return t[:].rearrange("p c (w j) -> p c w j", w=NCH)

    def _ro(t):
        # row-ordered tile [P, w, 2*j+c] -> (c, w, j) view
        return t[:].rearrange("p w (j c) -> p c w j", c=NCH)

    # ---------------- mask transpose on PE + scatter copy out ---------
    tbT_ps = psum.tile([P, NCH, 256], BF16, tag="tbT_ps")
    for c in range(NCH):
        for w in range(NCH):
            nc.tensor.transpose(tbT_ps[:, c, 128 * w:128 * (w + 1)],
                                tb[:, c, 128 * w:128 * (w + 1)], ident[:])
    tbT = pool.tile([P, NCH, 256], BF16, tag="tbT")
    nc.vector.tensor_scalar_add(_ro(tbT), _il(tbT_ps), 0.0)

    # ---------------- hrow transpose on PE ----------------------------
    uT_ps = psum.tile([P, NCH, 256], BF16, tag="uT_ps")
    for c in range(NCH):
        for w in range(NCH):
            nc.tensor.transpose(uT_ps[:, c, 128 * w:128 * (w + 1)],
                                U[:, c, 128 * w:128 * (w + 1)], ident[:])

    # ---------------- sigmoid + its transpose (unsigned) --------------
    sg = pool.tile([P, NCH, 256], BF16, tag="sg")
    sgT_ps = psum.tile([P, NCH, 256], BF16, tag="sgT_ps")
    for c in range(NCH):
        nc.scalar.activation(sg[:, c, :], Pt[:, c, :], Act.Sigmoid)
        for w in range(NCH):
            nc.tensor.transpose(sgT_ps[:, c, 128 * w:128 * (w + 1)],
                                sg[:, c, 128 * w:128 * (w + 1)], ident[:])

    # ---------------- squares + q tiles (ACT) -------------------------
    p2segs = _seg2(p2, LP, SEGSTR)
    p2il = (p2[:, LP:LP + 2 * SEGSTR]
            .rearrange("p (w x) -> p w x", w=NCH)[:, :, 0:256]
            .rearrange("p w (j c) -> p c w j", c=NCH))
    nc.scalar.activation(p2il, _il(uT_ps), Act.Square, bias=1.0, scale=1.0)

    qk = {}
    if K == 3:
        ks = [1, 2]
    else:
        ks = list(range(1, (K + 1 if K >= 255 else K)))
    for k in ks:
        if k * k not in qk:
            qx = pool.tile([P, PW], dt_e, tag=f"q{k * k}")
            nc.scalar.activation(qx[:], p2[:], Act.Copy, bias=float(k * k),
                                 scale=1.0)
            qk[k * k] = qx

    # ---------------- chain B equality + scans (DVE, per half) --------
    nc.vector.tensor_tensor(_seg2(YB, 1, 256, 255), tbT[:, :, 0:255],
                            tbT[:, :, 1:256], op=Alu.is_equal)
    FB = pool.tile([P, YW], BF16, tag="FB")
    BB = pool.tile([P, YW + 1], BF16, tag="BB")
    UC = pool.tile([P, NCH, 256], BF16, tag="UC")
    c2 = pool.tile([P, NCH, 256], dt_e, tag="c2")
    for w in range(NCH):
        lo = 256 * w
        nc.vector.tensor_tensor_scan(
            out=FB[:, lo:lo + 256], data0=ONEB[:, lo:lo + 256],
            data1=YB[:, lo:lo + 256], initial=BIG, op0=Alu.add, op1=Alu.mult)
        nc.vector.tensor_tensor_scan(
            out=BB[:, lo + 257:lo + 1:-1], data0=ONEB[:, lo + 256:lo:-1],
            data1=YB[:, lo + 256:lo:-1], initial=BIG, op0=Alu.add,
            op1=Alu.mult)
        nc.vector.tensor_tensor(UC[:, w, :], FB[:, lo:lo + 256],
                                BB[:, lo + 2:lo + 258], op=Alu.min)
        nc.scalar.activation(c2[:, w, :], UC[:, w, :], Act.Square,
                             bias=1.0, scale=1.0)
        if w == 0 and qk:
            # trigger the sqrt-set table load here: anchored on q1 so it
            # can neither be hoisted into the early table sequence nor
            # drift past the E ring onto the sqrt critical path
            qa = qk[min(qk)]
            nc.scalar.activation(dummy2[:], qa[0:1, 0:8], Act.Sqrt)

    # signed transposed sigmoid weight: wgt = sgT * (1 - 2*tbT)
    # ts1 = -2*tbT + 1 computed on ACT (Copy with scale/bias) to keep it
    # off the DVE critical path
    ts1 = pool.tile([P, NCH, 256], BF16, tag="ts1")
    nc.scalar.activation(ts1[:], tbT[:], Act.Copy, bias=1.0, scale=-2.0)
    wgt = pool.tile([P, NCH, 256], BF16, tag="wgt")
    nc.vector.tensor_tensor(_ro(wgt), _il(sgT_ps), _ro(ts1), op=Alu.mult)

    # ---------------- E = windowed parabola + ucol^2 ------------------
    E = pool.tile([P, NCH, 256], dt_e, tag="E")
    SUMS = pool.tile([P, 2], F32, tag="SUMS")
    MAXE = pool.tile([P, 2], F32, tag="MAXE")
    S = pool.tile([P, NCH, 256], BF16, tag="S")
    if K == 3:
        t1 = pool.tile([P, NCH, 256], dt_e, tag="t1")
        t2 = pool.tile([P, NCH, 256], dt_e, tag="t2")
        u1 = pool.tile([P, NCH, 256], dt_e, tag="u1")
        q1, q4 = qk[1], qk[4]
        # full-width ring prefix (fewer instruction overheads), per-half
        # finish so sqrt-w0 starts before the w1 mins complete
        nc.vector.tensor_tensor(t1[:], _seg2(q1, LP + 1, SEGSTR),
                                _seg2(q1, LP - 1, SEGSTR), op=Alu.min)
        nc.vector.tensor_tensor(t2[:], _seg2(q4, LP + 2, SEGSTR),
                                _seg2(q4, LP - 2, SEGSTR), op=Alu.min)
        nc.vector.tensor_tensor(u1[:], p2segs, t1[:], op=Alu.min)
        for w in range(NCH):
            nc.vector.tensor_tensor(t2[:, w, :], t2[:, w, :], c2[:, w, :],
                                    op=Alu.min)
            nc.vector.tensor_tensor(E[:, w, :], u1[:, w, :], t2[:, w, :],
                                    op=Alu.min)
            nc.scalar.sqrt(S[:, w, :], E[:, w, :])
            nc.vector.tensor_reduce(MAXE[:, w:w + 1], E[:, w, :],
                                    axis=mybir.AxisListType.XY, op=Alu.max)
    else:
        first = True
        for k in ks:
            for d in (k, -k):
                view = _seg2(qk[k * k], LP + d, SEGSTR)
                nc.vector.tensor_tensor(E[:], p2segs if first else E[:], view,
                                        op=Alu.min)
                first = False
        nc.vector.tensor_tensor(E[:], E[:], c2[:], op=Alu.min)
        for w in range(NCH):
            nc.scalar.sqrt(S[:, w, :], E[:, w, :])
            nc.vector.tensor_reduce(MAXE[:, w:w + 1], E[:, w, :],
                                    axis=mybir.AxisListType.XY, op=Alu.max)

    # ---------------- max fold (Pool, overlaps the dump) --------------
    FOLD = pool.tile([P, 4], F32, tag="FOLD")
    if POOL_FOLDS:
        nc.gpsimd.partition_all_reduce(FOLD[:, 2:4], MAXE[:], P,
                                       bass_isa.ReduceOp.max)

    # ---------------- signed dump -------------------------------------
    dump = pool.tile([P, NCH, 256], BF16, tag="dump")
    for w in range(NCH):
        nc.vector.scalar_tensor_tensor(
            dump[:, w, :], S[:, w, :], 0.0, wgt[:, w, :], op0=Alu.bypass,
            op1=Alu.mult, accum_out=SUMS[:, w:w + 1])

    # ---------------- output fold + DMA -------------------------------
    if POOL_FOLDS:
        nc.gpsimd.partition_all_reduce(FOLD[:, 0:2], SUMS[:], P,
                                       bass_isa.ReduceOp.add)
        nc.sync.dma_start(out_ap, FOLD[0:1, :], single_packet=True)
    else:
        nc.sync.dma_start(out_ap[:, 0:2], SUMS[:], single_packet=True)
        nc.sync.dma_start(out_ap[:, 2:4], MAXE[:], single_packet=True)


def build(K: int) -> bass.Bass:
    nc = bacc.Bacc("TRN2", target_bir_lowering=False, debug=False,
                   enable_asserts=False, num_devices=B)
    tgt_d = nc.dram_tensor("targets", [H, W], I32, kind="ExternalInput")
    pred_d = nc.dram_tensor("predictions", [H, W], F32, kind="ExternalInput")
    if POOL_FOLDS:
        out_d = nc.dram_tensor("out", [1, 4], F32, kind="ExternalOutput")
    else:
        out_d = nc.dram_tensor("out", [P, 4], F32, kind="ExternalOutput")
    with tile.TileContext(nc) as tc:
        with ExitStack() as ctx:
            _kernel_body(ctx, tc, out_d.ap(), tgt_d.ap(), pred_d.ap(), K)
    nc.compile()
    return nc


_nc_cache: dict = {}
LAST_K = 3


def _run(predictions: np.ndarray, targets: np.ndarray, K: int, trace=False):
    if K not in _nc_cache:
        _nc_cache[K] = build(K)
    nc = _nc_cache[K]
    in_maps = [
        {
            "targets": np.ascontiguousarray(targets[b, 0]),
            "predictions": np.ascontiguousarray(predictions[b, 0]),
        }
        for b in range(B)
    ]
    res = run_bass_kernel_spmd(nc, in_maps, core_ids=list(range(B)), trace=trace)
    outs = np.stack([r["out"] for r in res.results])
    return outs, res


def _reduce_outs(outs):
    """outs -> (ssum[B], maxE[B]) for both output layouts."""
    if POOL_FOLDS:
        ssum = outs[:, 0, 0:2].sum(axis=1, dtype=np.float64)
        maxE = outs[:, 0, 2:4].max(axis=1)
    else:
        ssum = outs[:, :, 0:2].sum(axis=(1, 2), dtype=np.float64)
        maxE = outs[:, :, 2:4].max(axis=(1, 2))
    return ssum, maxE


def _host_reference_sample(t2d, pred2d):
    """Exact numpy port of the reference for one sample (fallback path)."""
    BIGF = float(H + W)
    m = t2d != 0

    def dist1d_h(feat):
        out = np.empty((H, W), np.float64)
        d = np.full(W, BIGF)
        for i in range(H):
            d = np.where(feat[i], 0.0, d + 1.0)
            out[i] = d
        d = np.full(W, BIGF)
        for i in range(H - 1, -1, -1):
            d = np.where(feat[i], 0.0, d + 1.0)
            out[i] = np.minimum(out[i], d)
        return out

    def edt(feat):
        g = np.minimum(dist1d_h(feat), BIGF)
        g2 = g * g
        j = np.arange(W, dtype=np.float64)
        offs = (j[:, None] - j[None, :]) ** 2
        d2 = np.min(g2[:, None, :] + offs[None, :, :], axis=-1)
        return np.sqrt(d2)

    phi = edt(m) - edt(~m)
    denom = np.abs(phi).max() + 1e-8
    if not m.any():
        return 0.0
    sig = 1.0 / (1.0 + np.exp(-pred2d.astype(np.float64)))
    return float((phi / denom * sig).sum())


def kernel(predictions: np.ndarray, targets: np.ndarray) -> np.ndarray:
    global LAST_K
    predictions = np.asarray(predictions, dtype=np.float32)
    targets = np.asarray(targets, dtype=np.int32)

    fg = targets[:, 0] != 0
    nfg = fg.reshape(B, -1).sum(axis=1)
    has_fg = nfg > 0
    mixed = (nfg > 0) & (nfg < H * W)

    ki = 0
    while True:
        K = _K_LADDER[ki]
        outs, _ = _run(predictions, targets, K)
        ssum, maxE = _reduce_outs(outs)
        if K >= 255 or not mixed.any() or maxE[mixed].max() <= K * K:
            break
        need = np.sqrt(float(maxE[mixed].max()))
        ki += 1
        while ki < len(_K_LADDER) - 1 and _K_LADDER[ki] < need:
            ki += 1
    LAST_K = K

    total = 0.0
    for b in range(B):
        if not has_fg[b]:
            continue
        if not mixed[b]:
            total += _host_reference_sample(targets[b, 0], predictions[b, 0])
        else:
            denom = np.sqrt(float(maxE[b])) + 1e-8
            total += ssum[b] / denom
    return np.float32(total / (B * C * H * W))


if __name__ == "__main__":
    pred = np.load("/tmp/pred.npy")
    tgt = np.load("/tmp/tgt.npy")
    val = kernel(predictions=pred, targets=tgt)
    print("kernel loss:", repr(val))


# revision 32
# speedup vs baseline: 1.0497x; 1.0497x over previous
"""Boundary-loss kernel for Trainium2 (8 NeuronCores, pure data parallel).

Computes mean(phi_G * sigmoid(predictions)) where phi_G is the per-sample
normalized signed EDT of the target mask, via phi = (1-2t) * u2d with
u2d = distance to the nearest opposite-class pixel:

    u2d(r,c)^2 = min( (ucol+1)^2,  min_{|k|<K} (hrow(r+k,c)+1)^2 + k^2 )

hrow / ucol are exact 1-D opposite-distances (minus 1) along rows /
columns, from tensor_tensor_scan passes over equality fields.  Only the
vertical parabola is windowed (K), certified by max(E) <= K^2.

v2 engine plan (Pool engine supports only memset/iota/affine_select and
cross-partition reduces through this toolchain):
  DVE   : eq fields, all 6 scans, merges, ring mins, signed dump
  ACT   : casts, tbT PSUM->SBUF copy, sigmoid, squares, q=p2+k^2 biased
          copies, sqrt
  PE    : mask transpose, hrow transpose, sigmoid transpose (no DMA
          transposes at all)
  Pool  : memsets + cross-partition output folds (single-descriptor out)
  DMA   : inputs split across both HWDGE queues; output is [1,8] from
          partition 0 (one descriptor).
"""

import numpy as np
from contextlib import ExitStack

import concourse.bass as bass
import concourse.tile as tile
from concourse import bacc, mybir, masks, bass_isa
from concourse.bass_utils import run_bass_kernel_spmd

B, C, H, W = 8, 1, 256, 256
P = 128
NCH = 2
BIG = 300.0
PADV = 60000.0
YW = 2 * 256 + 1          # 513

Alu = mybir.AluOpType
Act = mybir.ActivationFunctionType
F32 = mybir.dt.float32
BF16 = mybir.dt.bfloat16
I32 = mybir.dt.int32

_K_LADDER = [3, 7, 15, 31, 63, 127, 255]
POOL_FOLDS = True         # gpsimd cross-partition reduces for the output


def _seg2(ap_tile, start, segstr, width=256):
    return (ap_tile[:, start:start + 2 * segstr]
            .rearrange("p (s t) -> p s t", s=2)[:, :, 0:width])


def _segw(ap_tile, start, segstr, w, width=256):
    return ap_tile[:, start + w * segstr: start + w * segstr + width]


def _kernel_body(ctx: ExitStack, tc, out_ap, tgt_ap, pred_ap, K: int):
    nc = tc.nc
    use_bf16 = K <= 11
    dt_e = BF16 if use_bf16 else F32

    SEGSTR = 256 + 2 * K + 2
    LP = K + 1
    PW = LP + 2 * SEGSTR + K + 2

    pool = ctx.enter_context(tc.tile_pool(name="work", bufs=1))
    psum = ctx.enter_context(tc.tile_pool(name="ps", bufs=1, space="PSUM"))

    # ---------------- input DMAs (halves on both HWDGE queues) --------
    T = pool.tile([P, NCH, 256], I32, tag="T")
    Pt = pool.tile([P, NCH, 256], F32, tag="Pt")
    # row r = c*128 + p; half c=0 rows 0..127, half c=1 rows 128..255.
    # targets split into quarters across both HWDGE queues so the first
    # eq/scan work can start ~1.2us earlier
    nc.sync.dma_start(T[0:64, 0, :], tgt_ap[0:64, :])
    nc.scalar.dma_start(T[64:P, 0, :], tgt_ap[64:P, :])
    nc.sync.dma_start(T[0:64, 1, :], tgt_ap[P:P + 64, :])
    nc.scalar.dma_start(T[64:P, 1, :], tgt_ap[P + 64:2 * P, :])
    nc.sync.dma_start(Pt[:, 0, :], pred_ap[0:P, :])
    nc.scalar.dma_start(Pt[:, 1, :], pred_ap[P:2 * P, :])

    # ---------------- constants (Pool) --------------------------------
    dummy1 = pool.tile([1, 8], F32, tag="dm1")
    dummy2 = pool.tile([1, 8], F32, tag="dm2")
    nc.gpsimd.memset(dummy1[:], 0.5)
    ONEB = pool.tile([P, YW], BF16, tag="ONEB")
    nc.gpsimd.memset(ONEB[:], 1.0)
    nc.gpsimd.memset(ONEB[:, 256:257], BIG)
    YA = pool.tile([P, YW], BF16, tag="YA")
    nc.gpsimd.memset(YA[:], 1.0)
    YB = pool.tile([P, YW], BF16, tag="YB")
    nc.gpsimd.memset(YB[:], 1.0)
    p2 = pool.tile([P, PW], dt_e, tag="p2")
    nc.gpsimd.memset(p2[:], PADV)
    ident = pool.tile([P, P], BF16, tag="ident")
    masks.make_identity(nc, ident[:])

    # ---------------- casts (ACT) -------------------------------------
    tb = pool.tile([P, NCH, 256], BF16, tag="tb")
    nc.scalar.activation(tb[:, 0, :], T[:, 0, :], Act.Copy)
    nc.scalar.activation(tb[:, 1, :], T[:, 1, :], Act.Copy)

    # ---------------- chain A equality + scans (DVE) ------------------
    for c in range(NCH):
        nc.vector.tensor_tensor(YA[:, 1 + 256 * c:256 * c + 256],
                                T[:, c, 0:255], T[:, c, 1:256],
                                op=Alu.is_equal)
    FA = pool.tile([P, YW], BF16, tag="FA")
    BA = pool.tile([P, YW + 1], BF16, tag="BA")
    for c in range(NCH):
        lo = 256 * c
        nc.vector.tensor_tensor_scan(
            out=FA[:, lo:lo + 256], data0=ONEB[:, lo:lo + 256],
            data1=YA[:, lo:lo + 256], initial=BIG, op0=Alu.add, op1=Alu.mult)
        nc.vector.tensor_tensor_scan(
            out=BA[:, lo + 257:lo + 1:-1], data0=ONEB[:, lo + 256:lo:-1],
            data1=YA[:, lo + 256:lo:-1], initial=BIG, op0=Alu.add,
            op1=Alu.mult)
    U = pool.tile([P, NCH, 256], BF16, tag="U")
    nc.vector.tensor_tensor(U[:], _seg2(FA, 0, 256), _seg2(BA, 2, 256),
                            op=Alu.min)

    # ---------------- mask transpose on PE + DVE copy out -------------
    tbT_ps = psum.tile([P, NCH, 256], BF16, tag="tbT_ps")
    for c in range(NCH):
        for w in range(NCH):
            nc.tensor.transpose(tbT_ps[:, w, 128 * c:128 * (c + 1)],
                                tb[:, c, 128 * w:128 * (w + 1)], ident[:])
    tbT = pool.tile([P, NCH, 256], BF16, tag="tbT")
    nc.vector.tensor_scalar_add(tbT[:], tbT_ps[:], 0.0)

    # ---------------- hrow transpose on PE ----------------------------
    uT_ps = psum.tile([P, NCH, 256], BF16, tag="uT_ps")
    for c in range(NCH):
        for w in range(NCH):
            nc.tensor.transpose(uT_ps[:, w, 128 * c:128 * (c + 1)],
                                U[:, c, 128 * w:128 * (w + 1)], ident[:])

    # ---------------- sigmoid + its transpose (unsigned) --------------
    sg = pool.tile([P, NCH, 256], BF16, tag="sg")
    sgT_ps = psum.tile([P, NCH, 256], BF16, tag="sgT_ps")
    for c in range(NCH):
        nc.scalar.activation(sg[:, c, :], Pt[:, c, :], Act.Sigmoid)
        for w in range(NCH):
            nc.tensor.transpose(sgT_ps[:, w, 128 * c:128 * (c + 1)],
                                sg[:, c, 128 * w:128 * (w + 1)], ident[:])

    # ---------------- squares + q tiles (ACT) -------------------------
    p2segs = _seg2(p2, LP, SEGSTR)
    nc.scalar.activation(p2segs, uT_ps[:], Act.Square, bias=1.0, scale=1.0)

    qk = {}
    if K == 3:
        ks = [1, 2]
    else:
        ks = list(range(1, (K + 1 if K >= 255 else K)))
    for k in ks:
        if k * k not in qk:
            qx = pool.tile([P, PW], dt_e, tag=f"q{k * k}")
            nc.scalar.activation(qx[:], p2[:], Act.Copy, bias=float(k * k),
                                 scale=1.0)
            qk[k * k] = qx

    # ---------------- chain B equality + scans (DVE, per half) --------
    nc.vector.tensor_tensor(_seg2(YB, 1, 256, 255), tbT[:, :, 0:255],
                            tbT[:, :, 1:256], op=Alu.is_equal)
    FB = pool.tile([P, YW], BF16, tag="FB")
    BB = pool.tile([P, YW + 1], BF16, tag="BB")
    UC = pool.tile([P, NCH, 256], BF16, tag="UC")
    c2 = pool.tile([P, NCH, 256], dt_e, tag="c2")
    for w in range(NCH):
        lo = 256 * w
        nc.vector.tensor_tensor_scan(
            out=FB[:, lo:lo + 256], data0=ONEB[:, lo:lo + 256],
            data1=YB[:, lo:lo + 256], initial=BIG, op0=Alu.add, op1=Alu.mult)
        nc.vector.tensor_tensor_scan(
            out=BB[:, lo + 257:lo + 1:-1], data0=ONEB[:, lo + 256:lo:-1],
            data1=YB[:, lo + 256:lo:-1], initial=BIG, op0=Alu.add,
            op1=Alu.mult)
        nc.vector.tensor_tensor(UC[:, w, :], FB[:, lo:lo + 256],
                                BB[:, lo + 2:lo + 258], op=Alu.min)
        nc.scalar.activation(c2[:, w, :], UC[:, w, :], Act.Square,
                             bias=1.0, scale=1.0)
        if w == 0 and qk:
            # trigger the sqrt-set table load here: anchored on q1 so it
            # can neither be hoisted into the early table sequence nor
            # drift past the E ring onto the sqrt critical path
            qa = qk[min(qk)]
            nc.scalar.activation(dummy2[:], qa[0:1, 0:8], Act.Sqrt)

    # signed transposed sigmoid weight: wgt = sgT * (1 - 2*tbT)
    # ts1 = -2*tbT + 1 computed on ACT (Copy with scale/bias) to keep it
    # off the DVE critical path
    ts1 = pool.tile([P, NCH, 256], BF16, tag="ts1")
    nc.scalar.activation(ts1[:], tbT[:], Act.Copy, bias=1.0, scale=-2.0)
    wgt = pool.tile([P, NCH, 256], BF16, tag="wgt")
    nc.vector.tensor_tensor(wgt[:], sgT_ps[:], ts1[:], op=Alu.mult)

    # ---------------- E = windowed parabola + ucol^2 ------------------
    E = pool.tile([P, NCH, 256], dt_e, tag="E")
    SUMS = pool.tile([P, 2], F32, tag="SUMS")
    MAXE = pool.tile([P, 2], F32, tag="MAXE")
    S = pool.tile([P, NCH, 256], BF16, tag="S")
    if K == 3:
        t1 = pool.tile([P, NCH, 256], dt_e, tag="t1")
        t2 = pool.tile([P, NCH, 256], dt_e, tag="t2")
        u1 = pool.tile([P, NCH, 256], dt_e, tag="u1")
        q1, q4 = qk[1], qk[4]
        # full-width ring prefix (fewer instruction overheads), per-half
        # finish so sqrt-w0 starts before the w1 mins complete
        nc.vector.tensor_tensor(t1[:], _seg2(q1, LP + 1, SEGSTR),
                                _seg2(q1, LP - 1, SEGSTR), op=Alu.min)
        nc.vector.tensor_tensor(t2[:], _seg2(q4, LP + 2, SEGSTR),
                                _seg2(q4, LP - 2, SEGSTR), op=Alu.min)
        nc.vector.tensor_tensor(u1[:], p2segs, t1[:], op=Alu.min)
        for w in range(NCH):
            nc.vector.tensor_tensor(t2[:, w, :], t2[:, w, :], c2[:, w, :],
                                    op=Alu.min)
            nc.vector.tensor_tensor(E[:, w, :], u1[:, w, :], t2[:, w, :],
                                    op=Alu.min)
            nc.scalar.sqrt(S[:, w, :], E[:, w, :])
            nc.vector.tensor_reduce(MAXE[:, w:w + 1], E[:, w, :],
                                    axis=mybir.AxisListType.XY, op=Alu.max)
    else:
        first = True
        for k in ks:
            for d in (k, -k):
                view = _seg2(qk[k * k], LP + d, SEGSTR)
                nc.vector.tensor_tensor(E[:], p2segs if first else E[:], view,
                                        op=Alu.min)
                first = False
        nc.vector.tensor_tensor(E[:], E[:], c2[:], op=Alu.min)
        for w in range(NCH):
            nc.scalar.sqrt(S[:, w, :], E[:, w, :])
            nc.vector.tensor_reduce(MAXE[:, w:w + 1], E[:, w, :],
                                    axis=mybir.AxisListType.XY, op=Alu.max)

    # ---------------- max fold (Pool, overlaps the dump) --------------
    FOLD = pool.tile([P, 4], F32, tag="FOLD")
    if POOL_FOLDS:
        nc.gpsimd.partition_all_reduce(FOLD[:, 2:4], MAXE[:], P,
                                       bass_isa.ReduceOp.max)

    # ---------------- signed dump -------------------------------------
    dump = pool.tile([P, NCH, 256], BF16, tag="dump")
    for w in range(NCH):
        nc.vector.scalar_tensor_tensor(
            dump[:, w, :], S[:, w, :], 0.0, wgt[:, w, :], op0=Alu.bypass,
            op1=Alu.mult, accum_out=SUMS[:, w:w + 1])

    # ---------------- output fold + DMA -------------------------------
    if POOL_FOLDS:
        nc.gpsimd.partition_all_reduce(FOLD[:, 0:2], SUMS[:], P,
                                       bass_isa.ReduceOp.add)
        nc.sync.dma_start(out_ap, FOLD[0:1, :], single_packet=True)
    else:
        nc.sync.dma_start(out_ap[:, 0:2], SUMS[:], single_packet=True)
        nc.sync.dma_start(out_ap[:, 2:4], MAXE[:], single_packet=True)


def build(K: int) -> bass.Bass:
    nc = bacc.Bacc("TRN2", target_bir_lowering=False, debug=False,
                   enable_asserts=False, num_devices=B)
    tgt_d = nc.dram_tensor("targets", [H, W], I32, kind="ExternalInput")
    pred_d = nc.dram_tensor("predictions", [H, W], F32, kind="ExternalInput")
    if POOL_FOLDS:
        out_d = nc.dram_tensor("out", [1, 4], F32, kind="ExternalOutput")
    else:
        out_d = nc.dram_tensor("out", [P, 4], F32, kind="ExternalOutput")
    with tile.TileContext(nc) as tc:
        with ExitStack() as ctx:
            _kernel_body(ctx, tc, out_d.ap(), tgt_d.ap(), pred_d.ap(), K)
    nc.compile()
    return nc


_nc_cache: dict = {}
LAST_K = 3


def _run(predictions: np.ndarray, targets: np.ndarray, K: int, trace=False):
    if K not in _nc_cache:
        _nc_cache[K] = build(K)
    nc = _nc_cache[K]
    in_maps = [
        {
            "targets": np.ascontiguousarray(targets[b, 0]),
            "predictions": np.ascontiguousarray(predictions[b, 0]),
        }
        for b in range(B)
    ]
    res = run_bass_kernel_spmd(nc, in_maps, core_ids=list(range(B)), trace=trace)
    outs = np.stack([r["out"] for r in res.results])
    return outs, res


def _reduce_outs(outs):
    """outs -> (ssum[B], maxE[B]) for both output layouts."""
    if POOL_FOLDS:
        ssum = outs[:, 0, 0:2].sum(axis=1, dtype=np.float64)
        maxE = outs[:, 0, 2:4].max(axis=1)
    else:
        ssum = outs[:, :, 0:2].sum(axis=(1, 2), dtype=np.float64)
        maxE = outs[:, :, 2:4].max(axis=(1, 2))
    return ssum, maxE


def _host_reference_sample(t2d, pred2d):
    """Exact numpy port of the reference for one sample (fallback path)."""
    BIGF = float(H + W)
    m = t2d != 0

    def dist1d_h(feat):
        out = np.empty((H, W), np.float64)
        d = np.full(W, BIGF)
        for i in range(H):
            d = np.where(feat[i], 0.0, d + 1.0)
            out[i] = d
        d = np.full(W, BIGF)
        for i in range(H - 1, -1, -1):
            d = np.where(feat[i], 0.0, d + 1.0)
            out[i] = np.minimum(out[i], d)
        return out

    def edt(feat):
        g = np.minimum(dist1d_h(feat), BIGF)
        g2 = g * g
        j = np.arange(W, dtype=np.float64)
        offs = (j[:, None] - j[None, :]) ** 2
        d2 = np.min(g2[:, None, :] + offs[None, :, :], axis=-1)
        return np.sqrt(d2)

    phi = edt(m) - edt(~m)
    denom = np.abs(phi).max() + 1e-8
    if not m.any():
        return 0.0
    sig = 1.0 / (1.0 + np.exp(-pred2d.astype(np.float64)))
    return float((phi / denom * sig).sum())


def kernel(predictions: np.ndarray, targets: np.ndarray) -> np.ndarray:
    global LAST_K
    predictions = np.asarray(predictions, dtype=np.float32)
    targets = np.asarray(targets, dtype=np.int32)

    fg = targets[:, 0] != 0
    nfg = fg.reshape(B, -1).sum(axis=1)
    has_fg = nfg > 0
    mixed = (nfg > 0) & (nfg < H * W)

    ki = 0
    while True:
        K = _K_LADDER[ki]
        outs, _ = _run(predictions, targets, K)
        ssum, maxE = _reduce_outs(outs)
        if K >= 255 or not mixed.any() or maxE[mixed].max() <= K * K:
            break
        need = np.sqrt(float(maxE[mixed].max()))
        ki += 1
        while ki < len(_K_LADDER) - 1 and _K_LADDER[ki] < need:
            ki += 1
    LAST_K = K

    total = 0.0
    for b in range(B):
        if not has_fg[b]:
            continue
        if not mixed[b]:
            total += _host_reference_sample(targets[b, 0], predictions[b, 0])
        else:
            denom = np.sqrt(float(maxE[b])) + 1e-8
            total += ssum[b] / denom
    return np.float32(total / (B * C * H * W))


if __name__ == "__main__":
    pred = np.load("/tmp/pred.npy")
    tgt = np.load("/tmp/tgt.npy")
    val = kernel(predictions=pred, targets=tgt)
    print("kernel loss:", repr(val))
